# revision 16
# baseline (speedup 1.0000x reference)
"""GAT (2-layer, PyG-style) on 8 Trainium2 NeuronCores via Bass/Tile.

v3: 2-row-packed SWDGE gathers. Q7 descriptor generation costs ~8ns/index
independent of element size, so the host pairs up graph nodes such that two
edges in the same destination window share one gather descriptor (elem =
2 rows). Phase 1 is replicated, so each core lays out its h_tab in its own
permutation (own dst nodes first, window-major; foreign nodes paired by
window-co-occurrence buckets). h2_tab order = concat of per-core own-node
orders; own-node pairing is chosen to maximize phase-3 packing. Outputs are
unpermuted on the host.

  - Phase 1 (replicated): h_ext = x_perm @ [W1 | W1@Asrc | W1@Adst] bf16 ->
    h_tab [npad, 384] (row: h(256 j-major) | aS(8) | aD(8) | pad).
  - Phase 2: per dst window (128 dsts): one packed gather (elem 768 = 2
    rows); self-loop rows via direct DMA (own block is h_tab[0:nchunk]);
    per-sub-chunk ST^T@adl broadcasts aD; segment softmax + S^T aggregation
    with denominator riding the p column; layer-2 row prep in-loop.
  - AllGather h2_mine -> h2_tab [n, 128] (row: h2(64) | aS2 | pad).
  - Phase 3: same machinery, single head, elem 256 = 2 rows; log_softmax.
"""
import sys

for _p in ("/opt/trn_rl_repo", "/opt/pypackages"):
    if _p not in sys.path:
        sys.path.insert(0, _p)

import numpy as np
from concourse import bacc, bass, mybir, tile
from concourse.masks import make_identity

P = 128
F32 = mybir.dt.float32
BF16 = mybir.dt.bfloat16
I16 = mybir.dt.int16

# ---- problem constants (nn_GAT_60000693125135) ----
N = 50000
IN_DIM = 256
H1 = 8
HID = 32
HC1 = H1 * HID  # 256
OUT = 64
NCORES = 8
NEG_SLOPE = 0.2
TROW = 384      # h_tab row stride (bf16 elems); cols 0:272 used
ECOL = HC1 + 2 * H1  # 272
L2C = 128       # h2_tab row stride (bf16); cols 0:65 used


def _cdiv(a, b):
    return -(-a // b)


def _wrap16(vals, nidx):
    a = np.asarray(vals, np.int16).reshape(nidx // 16, 16).T
    return np.tile(a, (8, 1))


# ----------------------------------------------------------------------------
# Host-side pairing + slot layout.
# ----------------------------------------------------------------------------
def _pair_by_buckets(node_groups, nodes):
    """Pair nodes with identical group-multisets; multi-round subset
    re-bucketing for the leftovers. Returns partner dict."""
    from collections import defaultdict
    buckets = defaultdict(list)
    for u in nodes:
        buckets[node_groups[u]].append(u)
    partner = {}
    leftovers = []
    for key, ns in buckets.items():
        k = len(ns)
        for i in range(0, k - (k % 2), 2):
            partner[ns[i]] = ns[i + 1]
            partner[ns[i + 1]] = ns[i]
        if k % 2:
            leftovers.append(ns[-1])
    # round 2: bucket leftovers by 2-subsets of their groups
    pend = defaultdict(list)
    for u in leftovers:
        if u in partner:
            continue
        key = node_groups[u]
        ks = sorted(set(key))[:4]
        done = False
        for i in range(len(ks)):
            for j in range(i + 1, len(ks)):
                sub = (ks[i], ks[j])
                while pend[sub] and pend[sub][-1] in partner:
                    pend[sub].pop()
                if pend[sub]:
                    v = pend[sub].pop()
                    partner[u] = v; partner[v] = u
                    done = True
                    break
            if done:
                break
        if not done:
            for i in range(len(ks)):
                for j in range(i + 1, len(ks)):
                    pend[(ks[i], ks[j])].append(u)
    # round 3: singles by single group
    pend1 = defaultdict(list)
    for u in leftovers:
        if u in partner:
            continue
        done = False
        for g in sorted(set(node_groups[u])):
            while pend1[g] and pend1[g][-1] in partner:
                pend1[g].pop()
            if pend1[g]:
                v = pend1[g].pop()
                partner[u] = v; partner[v] = u
                done = True
                break
        if not done:
            for g in set(node_groups[u]):
                pend1[g].append(u)
    return partner


def _pair_dense(counts, order_by):
    """Greedy max-overlap pairing from a dense count matrix.
    counts: [n, ncells] int8 (clipped at 3). Returns partner array (-1=none).
    Overlap(u,v) = sum_k min(cu,cv) computed via thresholded dot products."""
    n = counts.shape[0]
    a1 = (counts >= 1).astype(np.float32)
    a2 = (counts >= 2).astype(np.float32)
    a3 = (counts >= 3).astype(np.float32)
    OV = a1 @ a1.T + a2 @ a2.T + a3 @ a3.T
    np.fill_diagonal(OV, -1.0)
    partner = np.full(n, -1, np.int64)
    active = np.ones(n, bool)
    order = np.argsort(-order_by)
    for _ in range(40):
        todo = [u for u in order if active[u]]
        if len(todo) <= 1:
            break
        sub = OV[np.asarray(todo)]
        best = np.argmax(sub, axis=1)
        bestv = sub[np.arange(len(todo)), best]
        changed = False
        for i, u in enumerate(todo):
            if not active[u]:
                continue
            v = best[i]
            if bestv[i] <= 0:
                continue
            if not active[v]:
                continue   # partner taken this sweep; next round
            partner[u] = v; partner[v] = u
            active[u] = active[v] = False
            OV[:, u] = -1.0; OV[:, v] = -1.0
            changed = True
        if not changed:
            break
    # pair remaining actives arbitrarily
    rest = np.nonzero(active)[0]
    for i in range(0, len(rest) - (len(rest) % 2), 2):
        partner[rest[i]] = rest[i + 1]
        partner[rest[i + 1]] = rest[i]
    return partner


def _layout_slots(e_src_pos, e_dst128, nw, win_of_edge):
    """Vectorized: per window, pack edges into 2-row descriptors.
    Returns per-window (idx array, dsub [nd,2] array)."""
    pr = e_src_pos >> 1
    sub = e_src_pos & 1
    o = np.lexsort((sub, pr, win_of_edge))
    wo, pro, subo, do = win_of_edge[o], pr[o], sub[o], e_dst128[o]
    ne = len(o)
    # rank of edge within its (window, pair-row, sub) run
    grp = (wo * (1 << 40)) + (pro << 1) + subo
    new = np.ones(ne, bool)
    new[1:] = grp[1:] != grp[:-1]
    gid = np.cumsum(new) - 1
    starts = np.nonzero(new)[0]
    rank = np.arange(ne) - starts[gid]
    # per (window, pair-row): ndesc = max over sub of run length
    key2 = (wo * (1 << 40)) + (pro << 1)
    new2 = np.ones(ne, bool)
    new2[1:] = (key2[1:] >> 1) != (key2[:-1] >> 1)
    rid = np.cumsum(new2) - 1                   # (window,row) id
    nrun = np.zeros(rid[-1] + 1 if ne else 0, np.int64)
    np.maximum.at(nrun, rid, rank + 1)
    # desc base per (window,row), window-major
    row_win = wo[new2]
    row_pr = pro[new2]
    win_nd = np.zeros(nw, np.int64)
    np.add.at(win_nd, row_win, nrun)
    win_base = np.zeros(nw + 1, np.int64)
    np.cumsum(win_nd, out=win_base[1:])
    base = np.zeros(len(nrun), np.int64)
    # cumsum of nrun within each window
    cs = np.cumsum(nrun) - nrun
    woff = win_base[row_win]
    # cs counts globally; subtract window's global start
    win_cs0 = np.zeros(nw, np.int64)
    first_row = np.ones(len(nrun), bool)
    first_row[1:] = row_win[1:] != row_win[:-1]
    win_cs0[row_win[first_row]] = cs[first_row]
    base = cs - win_cs0[row_win] + woff
    desc = base[rid] + rank                     # global desc id (window-major)
    win_idx, win_d = [], []
    tot = int(win_base[nw])
    idx_flat = np.zeros(tot, np.int64)
    d_flat = np.full((tot, 2), -1, np.int64)
    idx_flat[desc] = pro
    d_flat[desc, subo] = do
    for x in range(nw):
        a, b = int(win_base[x]), int(win_base[x + 1])
        win_idx.append(idx_flat[a:b])
        win_d.append(d_flat[a:b])
    return win_idx, win_d


def prep_all(edge_index, n, ncores):
    """Build per-core permutations, packed gather indices, and one-hot
    S/ST operands for both phases."""
    import ml_dtypes
    bf16 = ml_dtypes.bfloat16
    from collections import defaultdict

    src = edge_index[0].astype(np.int64)
    dst = edge_index[1].astype(np.int64)
    nchunk = n // ncores
    nw = _cdiv(nchunk, P)
    core = dst // nchunk
    dloc = dst - core * nchunk
    w = dloc // P

    # ---- phase-3 cells: (core, window) of every edge, per source node ----
    ncell = ncores * nw
    cell = core * nw + w

    # ---- own-node pairing per core, WITHIN each natural window (keeps the
    # window membership stable so gather groups == profile cells) ----
    own_pos = np.empty(n, np.int64)
    for c in range(ncores):
        lo, hi = c * nchunk, (c + 1) * nchunk
        counts = np.zeros((nchunk, ncell + nw), np.int32)
        msrc = (src >= lo) & (src < hi)
        np.add.at(counts, (src[msrc] - lo, cell[msrc]), 1)
        mown = msrc & (core == c)
        np.add.at(counts, (src[mown] - lo, ncell + w[mown]), 1)
        cc = np.minimum(counts, 3).astype(np.int8)
        deg = counts.sum(axis=1)
        for x in range(nw):
            a, b = x * P, min((x + 1) * P, nchunk)
            partner = _pair_dense(cc[a:b], deg[a:b])
            order = []
            placed = np.zeros(b - a, bool)
            for ul in range(b - a):
                if placed[ul]:
                    continue
                v = partner[ul]
                if v >= 0 and not placed[v]:
                    order.append(ul); order.append(int(v))
                    placed[ul] = placed[v] = True
                else:
                    order.append(ul); placed[ul] = True
            own_pos[lo + a + np.asarray(order, np.int64)] = (
                a + np.arange(b - a))

    gpos = (np.arange(n) // nchunk) * nchunk + own_pos

    # ---- per-core phase-2 foreign pairing + positions ----
    ntiles = _cdiv(n, P)
    npad = ntiles * P
    perm_c = []
    for c in range(ncores):
        lo, hi = c * nchunk, (c + 1) * nchunk
        m = core == c
        sc, wc = src[m], w[m]
        o = np.lexsort((wc, sc))
        sco, wco = sc[o], wc[o]
        newn = np.ones(len(sco), bool)
        if len(sco):
            newn[1:] = sco[1:] != sco[:-1]
        bnds = np.nonzero(newn)[0]
        ng = {}
        for i, b in enumerate(bnds):
            e = bnds[i + 1] if i + 1 < len(bnds) else len(sco)
            u = int(sco[b])
            if lo <= u < hi:
                continue
            ng[u] = tuple(wco[b:e].tolist())
        foreign = list(ng)
        partner = _pair_by_buckets(ng, foreign)
        perm = np.full(n, -1, np.int64)
        perm[lo:hi] = own_pos[lo:hi]
        pos = nchunk
        for u in foreign:                      # paired first (even-aligned)
            if perm[u] >= 0:
                continue
            v = partner.get(u)
            if v is not None and perm[v] < 0:
                perm[u] = pos; perm[v] = pos + 1
                pos += 2
        for u in foreign:
            if perm[u] < 0:
                perm[u] = pos; pos += 1
        rest = np.nonzero(perm < 0)[0]
        perm[rest] = np.arange(pos, pos + len(rest))
        assert pos + len(rest) == n
        perm_c.append(perm)

    # ---- slot layouts (shared REG/chunk counts across cores) ----
    res = dict(perm_c=perm_c, own_pos=own_pos, nw=nw, npad=npad)
    iot = np.arange(P)
    for phase in (2, 3):
        wi_all, wd_all = [], []
        nd_all = np.zeros((ncores, nw), np.int64)
        for c in range(ncores):
            m = core == c
            s_c = src[m]
            d_c = own_pos[dst[m]]       # position within the core chunk
            e_pos = perm_c[c][s_c] if phase == 2 else gpos[s_c]
            wi, wd = _layout_slots(e_pos, d_c % P, nw, d_c // P)
            wi_all.append(wi); wd_all.append(wd)
            for x in range(nw):
                nd_all[c, x] = len(wi[x])
        REG = nd_all.max(axis=0)               # [nw] shared valid counts
        CK = np.maximum(_cdiv_arr(REG, P), 1)
        idx16, sts, ss = [], [], []
        for c in range(ncores):
            io, sto = [], []
            so = []
            for x in range(nw):
                nd = int(nd_all[c, x]); reg = int(REG[x]); ck = int(CK[x])
                iw = np.full(ck * P, -1, np.int64)
                iw[:nd] = wi_all[c][x]
                iw[nd:reg] = 0                 # dup-pad: valid fetch, dead S
                dpad = np.full((ck * P, 2), -1, np.int64)
                if nd:
                    dpad[:nd] = wd_all[c][x]
                dv = dpad.reshape(ck, P, 2).transpose(0, 2, 1).reshape(
                    ck * 2, P)
                io.append(iw)
                sto.append((dv[None, :, :] == iot[:, None, None]).reshape(
                    P, ck * 2 * P).astype(bf16))
                so.append(np.ascontiguousarray(
                    (dv[:, :, None] == iot[None, None, :]).transpose(
                        1, 0, 2).reshape(P, ck * 2 * P)).astype(bf16))
            iall = np.concatenate(io)
            idx16.append(_wrap16(iall, len(iall)))
            sts.append(np.concatenate(sto, axis=1))
            ss.append(np.concatenate(so, axis=1))
        res[f"idx{phase}"] = idx16
        res[f"st{phase}"] = sts
        res[f"s{phase}"] = ss
        res[f"ck{phase}"] = [int(v) for v in CK]
        res[f"reg{phase}"] = [int(v) for v in REG]
    return res


def _cdiv_arr(a, b):
    return -(-a // b)


# ----------------------------------------------------------------------------
# Kernel builder (SPMD program, same for all cores).
# ----------------------------------------------------------------------------
def build_nc(cfg):
    n = cfg["N"]; in_dim = cfg["IN"]; hc1 = cfg["HC1"]; h1 = cfg["H1"]
    hid = cfg["HID"]; out_dim = cfg["OUT"]; ncores = cfg["NCORES"]
    neg = cfg["NEG"]
    ck2, ck3 = cfg["ck2"], cfg["ck3"]
    reg2, reg3 = cfg["reg2"], cfg["reg3"]
    nw = cfg["nw"]; npad = cfg["npad"]

    nchunk = n // ncores
    ntiles = npad // P
    kt1 = _cdiv(in_dim, P)
    NB = 6
    cmax2 = max(ck2) + 1            # + self chunk
    cmax3 = max(ck3) + 1
    NIDX2 = sum(ck2) * P
    NIDX3 = sum(ck3) * P
    NSUB2 = 2 * sum(ck2)
    NSUB3 = 2 * sum(ck3)

    nc = bacc.Bacc(None, target_bir_lowering=False, debug=False,
                   num_devices=ncores)

    xT_in = nc.dram_tensor("xT", [in_dim, npad], BF16, kind="ExternalInput")
    w1e_in = nc.dram_tensor("W1ext", [in_dim, ECOL], BF16, kind="ExternalInput")
    w2e_in = nc.dram_tensor("W2ext", [hc1, out_dim + 2], BF16,
                            kind="ExternalInput")
    b1r_in = nc.dram_tensor("b1r", [P, hc1], F32, kind="ExternalInput")
    b2r_in = nc.dram_tensor("b2r", [P, out_dim], F32, kind="ExternalInput")
    idx2_in = nc.dram_tensor("idx2", [P, NIDX2 // 16], I16, kind="ExternalInput")
    idx3_in = nc.dram_tensor("idx3", [P, NIDX3 // 16], I16, kind="ExternalInput")
    st2_in = nc.dram_tensor("st2", [P, NSUB2 * P], BF16, kind="ExternalInput")
    s2_in = nc.dram_tensor("s2", [P, NSUB2 * P], BF16, kind="ExternalInput")
    st3_in = nc.dram_tensor("st3", [P, NSUB3 * P], BF16, kind="ExternalInput")
    s3_in = nc.dram_tensor("s3", [P, NSUB3 * P], BF16, kind="ExternalInput")
    out_ext = nc.dram_tensor("out", [nchunk, out_dim], F32,
                             kind="ExternalOutput")

    with tile.TileContext(nc) as tc:
        with (
            tc.tile_pool(name="dram", bufs=1, space="DRAM") as dram,
            tc.tile_pool(name="const", bufs=1) as cpool,
            tc.tile_pool(name="gbuf", bufs=3) as gpool,
            tc.tile_pool(name="g2buf", bufs=4) as g2pool,
            tc.tile_pool(name="stbuf", bufs=2) as stpool,
            tc.tile_pool(name="sbuf2", bufs=2) as spool,
            tc.tile_pool(name="small", bufs=3) as smpool,
            tc.tile_pool(name="psA", bufs=2, space="PSUM") as psA,
            tc.tile_pool(name="psB", bufs=2, space="PSUM") as psB,
            tc.tile_pool(name="psC", bufs=2, space="PSUM") as psC,
        ):
            h_tab = dram.tile([npad, TROW], BF16)
            h2_mine = dram.tile([nchunk, L2C], BF16)
            h2_tab = dram.tile([n, L2C], BF16, addr_space="Shared")

            identB = cpool.tile([P, P], BF16)
            make_identity(nc, identB[:])
            b1r = cpool.tile([P, hc1], F32)
            nc.sync.dma_start(out=b1r[:], in_=b1r_in[:])
            b2r = cpool.tile([P, out_dim], F32)
            nc.sync.dma_start(out=b2r[:], in_=b2r_in[:])
            w1e = cpool.tile([P, kt1, ECOL], BF16)
            for kt in range(kt1):
                kp = min(P, in_dim - kt * P)
                nc.sync.dma_start(out=w1e[:kp, kt, :],
                                  in_=w1e_in[kt * P:kt * P + kp, :])
            ckt = _cdiv(hc1, P)
            w2e = cpool.tile([P, ckt, out_dim + 2], BF16)
            for c in range(ckt):
                cp = min(P, hc1 - c * P)
                nc.sync.dma_start(out=w2e[:cp, c, :],
                                  in_=w2e_in[c * P:c * P + cp, :])
            zeros64 = cpool.tile([P, out_dim], F32)
            nc.vector.memset(zeros64[:], 0.0)
            zeros256 = cpool.tile([P, hc1], F32)
            nc.vector.memset(zeros256[:], 0.0)
            zband = cpool.tile([P, 2 * max(cmax2, cmax3), h1], BF16)
            nc.vector.memset(zband[:], 0.0)

            # ---- phase 1 ----
            with (
                tc.tile_pool(name="xst", bufs=2) as xpool,
                tc.tile_pool(name="hst", bufs=2) as hpool,
            ):
                for g in range(_cdiv(ntiles, NB)):
                    nt0 = g * NB
                    nb = min(NB, ntiles - nt0)
                    xst = xpool.tile([P, kt1, NB * P], BF16, tag="xst")
                    for kt in range(kt1):
                        kp = min(P, in_dim - kt * P)
                        nc.sync.dma_start(
                            out=xst[:kp, kt, 0:nb * P],
                            in_=xT_in[kt * P:kt * P + kp,
                                      nt0 * P:nt0 * P + nb * P])
                    hstg = hpool.tile([P, NB, ECOL], BF16, tag="hst")
                    for j in range(nb):
                        ps = psA.tile([P, ECOL], F32, tag="mm")
                        for kt in range(kt1):
                            kp = min(P, in_dim - kt * P)
                            nc.tensor.matmul(
                                out=ps[:], lhsT=xst[:kp, kt, j * P:(j + 1) * P],
                                rhs=w1e[:kp, kt, :],
                                start=(kt == 0), stop=(kt == kt1 - 1))
                        if j % 2 == 0:
                            nc.scalar.copy(out=hstg[:, j, :], in_=ps[:])
                        else:
                            nc.vector.tensor_copy(hstg[:, j, :], ps[:])
                    hv = h_tab[nt0 * P:(nt0 + nb) * P, 0:ECOL].rearrange(
                        "(j p) c -> p j c", p=P)
                    nc.sync.dma_start(out=hv, in_=hstg[:, 0:nb, :])

            # ---- own aD rows -> SBUF (own block = h_tab[0:nchunk]) ----
            adl = cpool.tile([P, nw, h1], BF16)
            a2l = cpool.tile([P, nw], BF16)
            nc.vector.memset(adl[:], 0.0)
            nc.vector.memset(a2l[:], 0.0)
            nwf = nchunk // P
            nc.sync.dma_start(
                out=adl[:, 0:nwf, :],
                in_=h_tab[0:nwf * P, hc1 + h1:ECOL].rearrange(
                    "(w p) c -> p w c", p=P))
            lrows = nchunk - nwf * P
            if lrows:
                nc.sync.dma_start(out=adl[:lrows, nwf, :],
                                  in_=h_tab[nwf * P:nchunk, hc1 + h1:ECOL])

            h_pairs = h_tab[:].rearrange("(a b) c -> a (b c)", b=2)
            h2_pairs = h2_tab[:].rearrange("(a b) c -> a (b c)", b=2)

            # ---- phase 2 ----
            ck2 = cfg["ck2"]
            off_i = 0    # idx col offset (in 16-wrapped cols)
            off_s = 0    # sub-chunk offset
            for i in range(3):
                gi = gpool.tile([P, cmax2, 2 * TROW], BF16, tag="G")
                nc.vector.memset(gi[:], 0.0)
            G_cur = gpool.tile([P, cmax2, 2 * TROW], BF16, tag="G")
            nc.scalar.copy(
                out=G_cur[:, 0:ck2[0] + 1, :].rearrange(
                    "p k (s c) -> p (k s) c", s=2)[:, :, hc1:hc1 + h1],
                in_=zband[:, 0:2 * (ck2[0] + 1), :])
            for x in range(nw):
                ck = ck2[x]
                nsub = 2 * ck
                rows = min(P, nchunk - x * P)
                G = G_cur
                idx2w = smpool.tile([P, cmax2 * 8], I16, tag="idx2w")
                nc.sync.dma_start(out=idx2w[:, 0:ck * 8],
                                  in_=idx2_in[:, off_i:off_i + ck * 8])
                nc.gpsimd.dma_gather(
                    out_ap=G[:, 0:ck, :], in_ap=h_pairs,
                    idxs_ap=idx2w[:, 0:ck * 8],
                    num_idxs=ck * P, num_idxs_reg=reg2[x],
                    elem_size=2 * TROW, single_packet=False)
                nc.sync.dma_start(
                    out=G[:rows, ck, 0:ECOL],
                    in_=h_tab[x * P:x * P + rows, 0:ECOL])
                if x + 1 < nw:
                    ckn = ck2[x + 1]
                    G_cur = gpool.tile([P, cmax2, 2 * TROW], BF16, tag="G")
                    nc.scalar.copy(
                        out=G_cur[:, 0:ckn + 1, :].rearrange(
                            "p k (s c) -> p (k s) c", s=2)[:, :, hc1:hc1 + h1],
                        in_=zband[:, 0:2 * (ckn + 1), :])
                STw = stpool.tile([P, 2 * cmax2, P], BF16, tag="ST")
                nc.sync.dma_start(out=STw[:, 0:nsub, :],
                                  in_=st2_in[:, P * off_s:P * (off_s + nsub)])
                S = spool.tile([P, 2 * cmax2, P], BF16, tag="S")
                nc.scalar.dma_start(out=S[:, 0:nsub, :],
                                    in_=s2_in[:, P * off_s:P * (off_s + nsub)])
                # aD broadcast per sub-chunk (+ self sub-chunk via identity)
                # ldweights-light aD broadcast: adl (8 cols) is the
                # loaded weight; each chunk's aD^T lands as [8, 128] rows at
                # base partition 32b (3 chunks/batch), then 32x32 DVE block
                # transposes flip back. Ascending order: a chunk's column
                # span is garbage-clobbered only by EARLIER chunks, then
                # overwritten by its own transposes (self col written last).
                aDsb = smpool.tile([P, 2 * cmax2 + 5, h1], BF16, tag="aDsb")
                aDflat = aDsb[:].rearrange("p k h -> p (k h)")
                for kb in range(0, nsub, 3):
                    m = min(3, nsub - kb)
                    aDT = psB.tile([96, P], F32, tag="aD")
                    t96 = smpool.tile([96, P], BF16, tag="t96")
                    for b in range(m):
                        nc.tensor.matmul(out=aDT[32 * b:32 * b + h1, :],
                                         lhsT=adl[:, x, :],
                                         rhs=STw[:, kb + b, :],
                                         start=True, stop=True)
                        nc.scalar.copy(out=t96[32 * b:32 * b + h1, :],
                                       in_=aDT[32 * b:32 * b + h1, :])
                    for b in range(m):
                        c0 = (kb + b) * h1
                        for q in range(4):
                            nc.vector.transpose(
                                out=aDflat[32 * q:32 * (q + 1), c0:c0 + 32],
                                in_=t96[32 * b:32 * (b + 1),
                                        32 * q:32 * (q + 1)])
                nc.vector.tensor_copy(aDsb[:, nsub, :], adl[:, x, :])
                # e = lrelu(aS + aD); p = exp(e) over the aS band (both subs
                # of edge chunks + sub 0 of the self chunk)
                easub = G[:, 0:ck + 1, :].rearrange(
                    "p k (s c) -> p (k s) c", s=2)[:, 0:nsub + 1, hc1:hc1 + h1]
                nc.vector.tensor_add(out=easub, in0=easub,
                                     in1=aDsb[:, 0:nsub + 1, :])
                nc.vector.scalar_tensor_tensor(
                    out=easub, in0=easub, scalar=neg, in1=easub,
                    op0=mybir.AluOpType.mult, op1=mybir.AluOpType.max)
                nc.scalar.activation(out=easub, in_=easub,
                                     func=mybir.ActivationFunctionType.Exp)
                # value cols are j-major: multiply by per-head p
                g4 = G[:, 0:ck + 1, :].rearrange(
                    "p k (s c) -> p (k s) c", s=2)[:, 0:nsub + 1, 0:hc1]
                g4r = g4.rearrange("p m (j h) -> p m j h", h=h1)
                nc.vector.tensor_tensor(
                    out=g4r, in0=g4r,
                    in1=G[:, 0:ck + 1, :].rearrange(
                        "p k (s c) -> p (k s) c", s=2)[
                        :, 0:nsub + 1, hc1:hc1 + h1].unsqueeze(2).to_broadcast(
                        (P, nsub + 1, hid, h1)),
                    op=mybir.AluOpType.mult)
                # aggregation (denominator rides the p column band)
                Gflat = G[:, 0:ck + 1, :].rearrange(
                    "p k (s c) -> p (k s) c", s=2)
                ops = psA.tile([P, hc1 + h1], F32, tag="mm")
                for k in range(nsub):
                    nc.tensor.matmul(out=ops[:], lhsT=S[:, k, :],
                                     rhs=Gflat[:, k, 0:hc1 + h1],
                                     start=(k == 0), stop=False)
                nc.tensor.matmul(out=ops[:], lhsT=identB[:],
                                 rhs=Gflat[:, nsub, 0:hc1 + h1],
                                 start=False, stop=True)
                rec = smpool.tile([P, h1], F32, tag="rec")
                nc.vector.reciprocal(out=rec[:], in_=ops[:, hc1:hc1 + h1])
                t1 = smpool.tile([P, hc1], F32, tag="t1")
                nc.vector.tensor_tensor(
                    out=t1[:].rearrange("p (j h) -> p j h", h=h1),
                    in0=ops[:, 0:hc1].rearrange("p (j h) -> p j h", h=h1),
                    in1=rec[:].unsqueeze(1).to_broadcast((P, hid, h1)),
                    op=mybir.AluOpType.mult)
                nc.vector.tensor_add(out=t1[:], in0=t1[:], in1=b1r[:])
                h1w = spool.tile([P, hc1], BF16, tag="h1w")
                nc.vector.tensor_tensor(out=h1w[:], in0=t1[:], in1=zeros256[:],
                                        op=mybir.AluOpType.max)
                h1T = spool.tile([P, ckt, P], BF16, tag="h1T")
                for c in range(ckt):
                    tp = psB.tile([P, P], BF16, tag="tp")
                    nc.tensor.transpose(tp[:], h1w[:, c * P:(c + 1) * P],
                                        identB[:])
                    nc.scalar.copy(out=h1T[:, c, :], in_=tp[:])
                h2ps = psC.tile([P, out_dim + 2], F32, tag="h2")
                for c in range(ckt):
                    nc.tensor.matmul(out=h2ps[:], lhsT=h1T[:, c, :],
                                     rhs=w2e[:, c, :],
                                     start=(c == 0), stop=(c == ckt - 1))
                h2sb = smpool.tile([P, out_dim + 2], BF16, tag="h2sb")
                nc.scalar.copy(out=h2sb[:], in_=h2ps[:])
                nc.sync.dma_start(
                    out=h2_mine[x * P:x * P + rows, 0:out_dim + 1],
                    in_=h2sb[:rows, 0:out_dim + 1])
                nc.scalar.copy(out=a2l[:rows, x:x + 1],
                               in_=h2ps[:rows, out_dim + 1:out_dim + 2])
                off_i += ck * 8
                off_s += nsub

            # ---- all-gather h2 ----
            nc.gpsimd.collective_compute(
                "AllGather", mybir.AluOpType.bypass,
                replica_groups=[list(range(ncores))],
                ins=[h2_mine[:].opt()], outs=[h2_tab[:].opt()])

            # ---- phase 3 ----
            ck3 = cfg["ck3"]
            t_all = cpool.tile([P, nw, out_dim], BF16)
            s_all = cpool.tile([P, nw], F32)
            off_i = 0
            off_s = 0
            for i in range(4):
                gi = g2pool.tile([P, cmax3, 2 * L2C], BF16, tag="G2")
                nc.vector.memset(gi[:], 0.0)
            G2_cur = g2pool.tile([P, cmax3, 2 * L2C], BF16, tag="G2")
            nc.scalar.copy(
                out=G2_cur[:, 0:ck3[0] + 1, :].rearrange(
                    "p k (s c) -> p (k s) c", s=2)[:, :, out_dim:out_dim + 1],
                in_=zband[:, 0:2 * (ck3[0] + 1), 0:1])
            for x in range(nw):
                ck = ck3[x]
                nsub = 2 * ck
                rows = min(P, nchunk - x * P)
                G2 = G2_cur
                idx3w = smpool.tile([P, cmax3 * 8], I16, tag="idx3w")
                nc.sync.dma_start(out=idx3w[:, 0:ck * 8],
                                  in_=idx3_in[:, off_i:off_i + ck * 8])
                nc.gpsimd.dma_gather(
                    out_ap=G2[:, 0:ck, :], in_ap=h2_pairs,
                    idxs_ap=idx3w[:, 0:ck * 8],
                    num_idxs=ck * P, num_idxs_reg=reg3[x],
                    elem_size=2 * L2C, single_packet=False)
                nc.sync.dma_start(
                    out=G2[:rows, ck, 0:out_dim + 1],
                    in_=h2_mine[x * P:x * P + rows, 0:out_dim + 1])
                if x + 1 < nw:
                    ckn = ck3[x + 1]
                    G2_cur = g2pool.tile([P, cmax3, 2 * L2C], BF16, tag="G2")
                    nc.scalar.copy(
                        out=G2_cur[:, 0:ckn + 1, :].rearrange(
                            "p k (s c) -> p (k s) c", s=2)[
                            :, :, out_dim:out_dim + 1],
                        in_=zband[:, 0:2 * (ckn + 1), 0:1])
                STw = stpool.tile([P, 2 * cmax3, P], BF16, tag="ST3")
                nc.sync.dma_start(out=STw[:, 0:nsub, :],
                                  in_=st3_in[:, P * off_s:P * (off_s + nsub)])
                S = spool.tile([P, 2 * cmax3, P], BF16, tag="S3")
                nc.scalar.dma_start(out=S[:, 0:nsub, :],
                                    in_=s3_in[:, P * off_s:P * (off_s + nsub)])
                aD2ps = psB.tile([P, 2 * cmax3], F32, tag="aD")
                for k in range(nsub):
                    nc.tensor.matmul(out=aD2ps[:, k:k + 1], lhsT=STw[:, k, :],
                                     rhs=a2l[:, x:x + 1], start=True, stop=True)
                aD2sb = smpool.tile([P, 2 * cmax3 + 1], BF16, tag="aD2sb")
                nc.scalar.copy(out=aD2sb[:, 0:nsub],
                               in_=aD2ps[:, 0:nsub])
                nc.vector.tensor_copy(aD2sb[:, nsub:nsub + 1],
                                      a2l[:, x:x + 1])
                easub = G2[:, 0:ck + 1, :].rearrange(
                    "p k (s c) -> p (k s) c", s=2)[
                    :, 0:nsub + 1, out_dim]
                nc.vector.tensor_add(out=easub, in0=easub,
                                     in1=aD2sb[:, 0:nsub + 1])
                nc.vector.scalar_tensor_tensor(
                    out=easub, in0=easub, scalar=neg, in1=easub,
                    op0=mybir.AluOpType.mult, op1=mybir.AluOpType.max)
                nc.scalar.activation(out=easub, in_=easub,
                                     func=mybir.ActivationFunctionType.Exp)
                G2flat = G2[:, 0:ck + 1, :].rearrange(
                    "p k (s c) -> p (k s) c", s=2)
                p2x = smpool.tile([P, 2 * cmax3 + 1, out_dim], BF16, tag="p2x")
                nc.scalar.copy(
                    out=p2x[:, 0:nsub + 1, :],
                    in_=G2flat[:, 0:nsub + 1, out_dim:out_dim + 1].to_broadcast(
                        (P, nsub + 1, out_dim)))
                nc.vector.tensor_tensor(
                    out=G2flat[:, 0:nsub + 1, 0:out_dim],
                    in0=G2flat[:, 0:nsub + 1, 0:out_dim],
                    in1=p2x[:, 0:nsub + 1, :],
                    op=mybir.AluOpType.mult)
                ops2 = psA.tile([P, out_dim + 1], F32, tag="mm")
                for k in range(nsub):
                    nc.tensor.matmul(out=ops2[:], lhsT=S[:, k, :],
                                     rhs=G2flat[:, k, 0:out_dim + 1],
                                     start=(k == 0), stop=False)
                nc.tensor.matmul(out=ops2[:], lhsT=identB[:],
                                 rhs=G2flat[:, nsub, 0:out_dim + 1],
                                 start=False, stop=True)
                rec2 = smpool.tile([P, 1], F32, tag="rec")
                nc.vector.reciprocal(out=rec2[:], in_=ops2[:, out_dim:])
                z = smpool.tile([P, out_dim], F32, tag="z")
                nc.vector.tensor_tensor(
                    out=z[:], in0=ops2[:, 0:out_dim],
                    in1=rec2[:].to_broadcast((P, out_dim)),
                    op=mybir.AluOpType.mult)
                nc.vector.tensor_add(out=z[:], in0=z[:], in1=b2r[:])
                negmax = smpool.tile([P, 1], F32, tag="nm")
                nc.vector.tensor_reduce(out=negmax[:], in_=z[:],
                                        axis=mybir.AxisListType.X,
                                        op=mybir.AluOpType.max, negate=True)
                nc.vector.scalar_tensor_tensor(
                    out=t_all[:, x, :], in0=z[:], scalar=negmax[:],
                    in1=zeros64[:],
                    op0=mybir.AluOpType.add, op1=mybir.AluOpType.add)
                esc = smpool.tile([P, out_dim], F32, tag="esc")
                nc.scalar.activation(out=esc[:], in_=z[:],
                                     func=mybir.ActivationFunctionType.Exp,
                                     bias=negmax[:],
                                     accum_out=s_all[:, x:x + 1])
                off_i += ck * 8
                off_s += nsub
            lns = cpool.tile([P, nw], F32)
            nc.scalar.activation(out=lns[:], in_=s_all[:],
                                 func=mybir.ActivationFunctionType.Ln)
            for x in range(nw):
                rows = min(P, nchunk - x * P)
                res = smpool.tile([P, out_dim], F32, tag="res")
                nc.vector.scalar_tensor_tensor(
                    out=res[:], in0=t_all[:, x, :], scalar=lns[:, x:x + 1],
                    in1=zeros64[:],
                    op0=mybir.AluOpType.subtract, op1=mybir.AluOpType.add)
                nc.sync.dma_start(out=out_ext[x * P:x * P + rows, :],
                                  in_=res[:rows, :])

    return nc


# ----------------------------------------------------------------------------
# Host-side input packing.
# ----------------------------------------------------------------------------
def make_in_maps(inputs, cfg):
    import ml_dtypes
    bf16 = ml_dtypes.bfloat16
    n = cfg["N"]; in_dim = cfg["IN"]; hc1 = cfg["HC1"]; h1 = cfg["H1"]
    hid = cfg["HID"]; out_dim = cfg["OUT"]; ncores = cfg["NCORES"]

    x = np.asarray(inputs["x"], np.float32)
    ei = np.asarray(inputs["edge_index"])
    W1 = np.asarray(inputs["W1"], np.float32)
    a_src1 = np.asarray(inputs["a_src1"], np.float32)
    a_dst1 = np.asarray(inputs["a_dst1"], np.float32)
    b1 = np.asarray(inputs["b1"], np.float32)
    W2 = np.asarray(inputs["W2"], np.float32)
    a_src2 = np.asarray(inputs["a_src2"], np.float32)
    a_dst2 = np.asarray(inputs["a_dst2"], np.float32)
    b2 = np.asarray(inputs["b2"], np.float32)

    amat = np.zeros((hc1, 2 * h1), np.float32)
    for h in range(h1):
        amat[h * hid:(h + 1) * hid, h] = a_src1[h]
        amat[h * hid:(h + 1) * hid, h1 + h] = a_dst1[h]
    jmaj = np.arange(hc1).reshape(hid, h1)
    perm = (jmaj % h1) * hid + jmaj // h1
    perm = perm.reshape(-1)
    W1ext = np.concatenate([W1[:, perm], W1 @ amat], axis=1).astype(bf16)
    W2e_full = np.concatenate(
        [W2, (W2 @ a_src2[0])[:, None], (W2 @ a_dst2[0])[:, None]], axis=1)
    W2ext = W2e_full[perm, :].astype(bf16)
    b1p = b1[perm]

    pe = prep_all(ei, n, ncores)
    for k in ("ck2", "ck3", "reg2", "reg3", "nw", "npad"):
        cfg[k] = pe[k]
    cfg["own_pos"] = pe["own_pos"]

    npad = pe["npad"]
    nw = pe["nw"]
    xb = x.astype(bf16)

    common = {
        "W1ext": W1ext, "W2ext": W2ext,
        "b1r": np.tile(b1p[None, :], (P, 1)).astype(np.float32),
        "b2r": np.tile(b2[None, :], (P, 1)).astype(np.float32),
    }
    in_maps = []
    for c in range(ncores):
        m = dict(common)
        permc = pe["perm_c"][c]
        xT = np.zeros((in_dim, npad), bf16)
        xT[:, permc] = xb.T
        m["xT"] = xT
        m["idx2"] = np.ascontiguousarray(pe["idx2"][c])
        m["idx3"] = np.ascontiguousarray(pe["idx3"][c])
        m["st2"] = np.ascontiguousarray(pe["st2"][c])
        m["s2"] = np.ascontiguousarray(pe["s2"][c])
        m["st3"] = np.ascontiguousarray(pe["st3"][c])
        m["s3"] = np.ascontiguousarray(pe["s3"][c])
        in_maps.append(m)
    return in_maps


DEFAULT_CFG = dict(N=N, IN=IN_DIM, HC1=HC1, H1=H1, HID=HID, OUT=OUT,
                   NCORES=NCORES, NEG=NEG_SLOPE)

TRACE = False
LAST_RESULTS = None


def kernel(**inputs) -> np.ndarray:
    global LAST_RESULTS
    from concourse.bass_utils import run_bass_kernel_spmd

    cfg = dict(DEFAULT_CFG)
    in_maps = make_in_maps(inputs, cfg)
    nc = build_nc(cfg)
    if not nc.is_finalized():
        nc.finalize()
    res = run_bass_kernel_spmd(nc, in_maps, core_ids=list(range(cfg["NCORES"])),
                               trace=TRACE)
    LAST_RESULTS = res
    outs = [res.results[c]["out"] for c in range(cfg["NCORES"])]
    raw = np.concatenate(outs, axis=0).astype(np.float32)
    # unpermute: output row (c, pos) holds the node with own_pos == pos
    n = cfg["N"]
    nchunk = n // cfg["NCORES"]
    own_pos = cfg["own_pos"]
    final = np.empty_like(raw)
    gidx = (np.arange(n) // nchunk) * nchunk + own_pos
    final[np.arange(n)] = raw[gidx]
    return final
